# revision 2
# baseline (speedup 1.0000x reference)
"""Trainium2 Bass kernel for nn_Graph_to_Featuremaps_savemem.

Math: the reference computes, per batch b,
    scores[b,p,n] = (res @ nfr)[b,p] + (x @ nfh)[b,n]
    attn = softmax_n(scores);  out[b,p,c] = (attn @ (x @ W))[b,p,c]
Softmax over n is shift-invariant, so the (res @ nfr)[b,p] term cancels:
    attn[b,p,:] = softmax(x[b] @ nfh)   (independent of p)
    out[b,c,h,w] = relu(((softmax(x[b]@nfh) @ x[b]) @ W)[c])   broadcast over (h,w)
res_feature never affects the output. The kernel is therefore a tiny per-batch
compute (one 64-softmax + two small matmuls) followed by a 256 MB broadcast
write — pure HBM-write-bound, sharded batch-parallel over 8 cores (2 batches,
32 MB written per core).

The 32 MiB/core write runs at the 16-SDMA-engine fabric ceiling (~420 GB/s),
so the optimization targets are (a) time-to-first-output-packet and (b)
descriptor size. Ramp is shortened by: a single packed input DMA (x, x^T, W,
nfh plus a block-diagonal mask and a ones panel in one [128,643] tile), a
host-side transpose instead of a PE transpose, batching both per-batch
reductions into single matmuls via the mask (E2 = e * mask is block-diagonal,
so X^T @ E2 gives both U_b columns and ones^T @ E2 both softmax sums), and a
2048-wide quick fill so the first output DMA issues right after the ~3 us
compute chain. The bulk of each 128-row output block is written by ONE
broadcast-read DMA whose SBUF-side access pattern has a stride-0 middle dim
([[p,128],[0,3],[1,4096]]) re-reading the same 16 KiB fill window — deep
queues from few instructions, 16 KiB descriptors.
"""

import numpy as np

N_CORES = 8
B, NODES, HID, C, H, W = 16, 64, 128, 256, 128, 128
HWP = H * W  # 16384
B_LOC = B // N_CORES  # 2 batches per core

# packed input column offsets
XC, XTC, WC, NFHC, MKC, ONC = 0, 128, 256, 512, 513, 515
PKW = 643

QW = 2048  # quick-fill width (8 KiB rows); widened to 2*QW for the bulk DMA

_NC_CACHE = {}


def build_nc():
    import concourse.bass as bass
    import concourse.bacc as bacc
    import concourse.mybir as mybir
    from concourse.tile import TileContext

    f32 = mybir.dt.float32
    Alu = mybir.AluOpType
    Act = mybir.ActivationFunctionType

    nc = bacc.Bacc(None, target_bir_lowering=False, debug=False)
    pk_d = nc.declare_dram_parameter("pk", [128, PKW], f32, isOutput=False)
    out_d = nc.declare_dram_parameter("out", [B_LOC * C, HWP], f32, isOutput=True)

    def bcast_mid(ap, reps):
        # (P,F) AP -> (P,reps,F) AP re-reading the same F-wide window
        return type(ap)(ap.tensor, ap.offset, [list(ap.ap[0]), [0, reps], list(ap.ap[1])])

    with TileContext(nc) as tc:
        with (
            tc.tile_pool(name="singles", bufs=1) as singles,
            tc.tile_pool(name="fills", bufs=1) as fills,
            tc.tile_pool(name="psum", bufs=1, space="PSUM") as psum,
        ):
            ZERO = singles.tile([128, QW], f32, tag="ZERO")
            nc.vector.memset(ZERO[:], 0.0)

            PK = singles.tile([128, PKW], f32, tag="PK")
            nc.sync.dma_start(out=PK[:], in_=pk_d[:])
            x_ap = PK[:, XC : XC + HID]           # (bn, hid)
            xt_ap = PK[:, XTC : XTC + B_LOC * NODES]  # (hid, bn)
            nfh_ap = PK[:, NFHC : NFHC + 1]       # (hid, 1)
            mask_ap = PK[:, MKC : MKC + B_LOC]    # (bn, 2) block-diagonal indicator
            onec_ap = PK[:, ONC : ONC + 1]        # (bn, 1) ones
            oner_ap = PK[0:1, ONC : ONC + 128]    # (1, 128) ones

            # s = x @ nfh  (bn, 1)
            s_ps = psum.tile([B_LOC * NODES, 1], f32, tag="s")
            nc.tensor.matmul(s_ps[:], xt_ap, nfh_ap)
            # e = exp(s)   (unnormalized softmax numerator; scores are O(1))
            e_sb = singles.tile([128, 1], f32, tag="e")
            nc.scalar.activation(e_sb[:], s_ps[:], Act.Exp)
            # E2[:, b] = e masked to batch b  -> one matmul gives both U_b and sum_b
            E2 = singles.tile([128, B_LOC], f32, tag="E2")
            nc.vector.tensor_scalar(E2[:], mask_ap, e_sb[:], None, op0=Alu.mult)
            U2_ps = psum.tile([HID, B_LOC], f32, tag="U2")
            nc.tensor.matmul(U2_ps[:], x_ap, E2[:])
            S_ps = psum.tile([1, B_LOC], f32, tag="S")
            nc.tensor.matmul(S_ps[:], onec_ap, E2[:])
            r_sb = singles.tile([1, B_LOC], f32, tag="r")
            nc.vector.reciprocal(r_sb[:], S_ps[:])
            # RC[p, b] = 1/sum_b on every partition
            RC_ps = psum.tile([128, B_LOC], f32, tag="RC")
            nc.tensor.matmul(RC_ps[:], oner_ap, r_sb[:])
            RC = singles.tile([128, B_LOC], f32, tag="RC_sb")
            nc.vector.tensor_copy(RC[:], RC_ps[:])
            U2 = singles.tile([HID, B_LOC], f32, tag="U2_sb")
            nc.vector.tensor_copy(U2[:], U2_ps[:])

            # V_h = W_h^T @ U2 (c-major half h), VR = V * (1/sum) per column
            VRs = []
            for h in range(C // 128):
                V_ps = psum.tile([128, B_LOC], f32, tag=f"V{h}")
                nc.tensor.matmul(V_ps[:], PK[:, WC + 128 * h : WC + 128 * (h + 1)], U2[:])
                VR = singles.tile([128, B_LOC], f32, tag=f"VR{h}")
                nc.vector.tensor_mul(VR[:], V_ps[:], RC[:])
                VRs.append(VR)

            # blocks ordered by fill readiness: h=0 first (VR0 ready earlier)
            reps = (HWP - 2 * QW) // (2 * QW)
            k = 0
            for h in range(C // 128):
                for b in range(B_LOC):
                    r0 = b * C + h * 128
                    fill = fills.tile([128, 2 * QW], f32, tag=f"fill{b}{h}")
                    # fill[:, :QW] = relu(VR[:,b]) broadcast along free dim
                    nc.vector.tensor_scalar(
                        fill[:, 0:QW], ZERO[:], VRs[h][:, b : b + 1], 0.0,
                        op0=Alu.add, op1=Alu.max,
                    )
                    eng = [nc.sync, nc.scalar]
                    eng[k % 2].dma_start(
                        out=out_d[r0 : r0 + 128, 0:QW], in_=fill[:, 0:QW]
                    )
                    # widen to 2*QW for 16 KiB descriptors on the bulk DMA
                    nc.vector.tensor_copy(fill[:, QW : 2 * QW], fill[:, 0:QW])
                    eng[(k + 1) % 2].dma_start(
                        out=out_d[r0 : r0 + 128, QW : 2 * QW],
                        in_=fill[:, QW : 2 * QW],
                    )
                    # bulk: one broadcast-read DMA re-reads the same 2*QW window
                    eng[k % 2].dma_start(
                        out=out_d[r0 : r0 + 128, 2 * QW : HWP],
                        in_=bcast_mid(fill[:, 0 : 2 * QW], reps),
                    )
                    k += 1
    nc.finalize()
    return nc


def get_nc():
    if "nc" not in _NC_CACHE:
        _NC_CACHE["nc"] = build_nc()
    return _NC_CACHE["nc"]


def make_in_maps(input, node_fea_for_hidden, weight):
    x = np.asarray(input, np.float32)[0]  # (B, NODES, HID)
    nfh = np.asarray(node_fea_for_hidden, np.float32).reshape(HID, 1)
    w = np.asarray(weight, np.float32)
    mask = np.zeros((B_LOC * NODES, B_LOC), np.float32)
    for b in range(B_LOC):
        mask[b * NODES : (b + 1) * NODES, b] = 1.0
    ones = np.ones((128, 128), np.float32)
    in_maps = []
    for i in range(N_CORES):
        xs = x[i * B_LOC : (i + 1) * B_LOC].reshape(B_LOC * NODES, HID)
        pk = np.concatenate(
            [xs, xs.T, w, nfh, mask, ones], axis=1, dtype=np.float32
        )
        assert pk.shape == (128, PKW), pk.shape
        in_maps.append({"pk": np.ascontiguousarray(pk)})
    return in_maps


def run_spmd(in_maps, trace=False, **kw):
    from concourse.bass_utils import run_bass_kernel_spmd

    return run_bass_kernel_spmd(get_nc(), in_maps, list(range(N_CORES)), trace=trace, **kw)


def kernel(input, res_feature, node_fea_for_res, node_fea_for_hidden, weight):
    res = run_spmd(make_in_maps(input, node_fea_for_hidden, weight)).results
    out = np.concatenate(
        [r["out"].reshape(B_LOC, C, H, W) for r in res], axis=0
    )
    return out
